# revision 4
# baseline (speedup 1.0000x reference)
"""Trainium2 Bass kernel for the decomposable-attention alignment model.

Data-parallel over batch across 8 NeuronCores (16 batch elements each).
All matmuls run as float32r (TF32-like: 1 cycle/row at free-dim>=256,
~1e-4 relative error). The compare GEMM packs two batch elements per
moving operand (N=512). Aggregation (sum over length after tanh) is
fused into the PSUM-drain activation via accum_out. The final MLP
streams W1/W2 in bf16 with biases folded in as extra contraction rows.
"""

import sys

sys.path.insert(0, "/opt/trn_rl_repo")

import numpy as np
import ml_dtypes

from concourse import bass, tile, mybir
from concourse.bass_utils import run_bass_kernel_spmd

F32 = mybir.dt.float32
F32R = mybir.dt.float32r
BF16 = mybir.dt.bfloat16

N_CORES = 8
B, L, E, A, FF = 128, 256, 512, 256, 2048
BL = B // N_CORES  # 16 batch elements per core
G = 4  # batch block size (pairs of 2)
ACT_F = mybir.ActivationFunctionType


def _split_multiwait(nc, max_waits=1):
    """walrus in this image rejects >1 sync wait per instruction; hoist
    extras onto InstNoOp placed just before the offender."""
    for f in nc.m.functions:
        for bb in f.blocks:
            out = []
            changed = False
            for inst in bb.instructions:
                si = inst.sync_info
                waits = list(si.on_wait) if (si is not None and si.on_wait) else []
                if len(waits) > max_waits:
                    extra, keep = waits[:-max_waits], waits[-max_waits:]
                    for j, w in enumerate(extra):
                        nop = mybir.InstNoOp(name=f"{inst.name}-ws{j}")
                        nop.engine = inst.engine
                        nop.sync_info = mybir.SyncInfo(on_wait=[w], on_update=[])
                        out.append(nop)
                    inst.sync_info = mybir.SyncInfo(
                        on_wait=keep, on_update=list(si.on_update or [])
                    )
                    changed = True
                out.append(inst)
            if changed:
                bb.instructions = out


def build_nc():
    nc = bass.Bass()

    p_nat = nc.declare_dram_parameter("p_nat", [BL, L, E], F32R, isOutput=False)
    h_nat = nc.declare_dram_parameter("h_nat", [BL, L, E], F32R, isOutput=False)
    pT = nc.declare_dram_parameter("pT", [BL, E, L], F32R, isOutput=False)
    hT = nc.declare_dram_parameter("hT", [BL, E, L], F32R, isOutput=False)
    wf_d = nc.declare_dram_parameter("wf", [E, A], F32R, isOutput=False)
    wg_d = nc.declare_dram_parameter("wg", [2 * E, FF], F32R, isOutput=False)
    w1_d = nc.declare_dram_parameter("w1", [2 * FF + 1, FF], BF16, isOutput=False)
    w2_d = nc.declare_dram_parameter("w2", [FF + 1, FF], BF16, isOutput=False)
    w3_d = nc.declare_dram_parameter("w3", [128, 68], F32R, isOutput=False)
    ident_d = nc.declare_dram_parameter("ident", [128, 128], F32R, isOutput=False)
    ones_bf_d = nc.declare_dram_parameter("ones_bf", [1, BL], BF16, isOutput=False)
    ones_r_d = nc.declare_dram_parameter("ones_r", [1, BL], F32R, isOutput=False)
    out_d = nc.declare_dram_parameter("out", [BL, 3], F32, isOutput=True)

    NKT_G = 2 * E // 128  # 8 contraction tiles for the compare GEMM
    NMT_G = FF // 128  # 16 output tiles

    with tile.TileContext(nc) as tc:
        with (
            tc.tile_pool(name="wpool", bufs=1) as wpool,
            tc.tile_pool(name="xt", bufs=1) as xtp,
            tc.tile_pool(name="ph", bufs=1) as php,
            tc.tile_pool(name="att", bufs=2) as attp,
            tc.tile_pool(name="vt", bufs=1) as vtp,
            tc.tile_pool(name="mlp", bufs=1) as mlpp,
            tc.tile_pool(name="w1s", bufs=2) as w1sp,
            tc.tile_pool(name="scrap", bufs=1) as scrapp,
            tc.tile_pool(name="psum", bufs=2, space="PSUM") as psp,
        )        :
            # ---- resident weights ----
            wg = [wpool.tile([128, FF], F32R, tag=f"wg{k}", name=f"wg{k}") for k in range(NKT_G)]
            for k in range(NKT_G):
                nc.sync.dma_start(wg[k][:], wg_d[128 * k : 128 * (k + 1), :])
            wf = [wpool.tile([128, A], F32R, tag=f"wf{k}", name=f"wf{k}") for k in range(4)]
            for k in range(4):
                nc.sync.dma_start(wf[k][:], wf_d[128 * k : 128 * (k + 1), :])
            idt = wpool.tile([128, 128], F32R, tag="idt", name="idt")
            nc.sync.dma_start(idt[:], ident_d[:])
            w3t = wpool.tile([128, 68], F32R, tag="w3t", name="w3t")
            nc.sync.dma_start(w3t[:], w3_d[:])

            # vT accumulator: 32 tiles [128, BL] fp32, c4 index = side*16 + fm
            vT = [vtp.tile([128, BL], F32, tag=f"vt{i}", name=f"vt{i}") for i in range(32)]

            scrap = scrapp.tile([128, L], BF16, tag="scrap", name="scrap")

            def q(i, shape, name):
                return psp.tile(shape, F32, tag=f"q{i}", name=name)

            def qr(i, shape, name):
                return psp.tile(shape, F32R, tag=f"q{i}", name=name)

            # ================= batch blocks =================
            for blk in range(BL // G):
                # xt[pair][side][ck]: [128, 512]; cols 0:256 = batch 2p,
                # cols 256: = batch 2p+1. ck 0..3 from pT/hT, 4..7 = betasT/alphasT
                xt = [
                    [
                        [
                            xtp.tile(
                                [128, 2 * L], F32R,
                                tag=f"xt{p}_{s}_{ck}", name=f"xt{p}_{s}_{ck}",
                            )
                            for ck in range(NKT_G)
                        ]
                        for s in range(2)
                    ]
                    for p in range(2)
                ]
                for gi in range(G):
                    b = blk * G + gi
                    pr, half = gi // 2, (gi % 2) * L
                    for et in range(4):
                        nc.sync.dma_start(
                            xt[pr][0][et][:, half : half + L],
                            pT[b][128 * et : 128 * (et + 1), :],
                        )
                        nc.sync.dma_start(
                            xt[pr][1][et][:, half : half + L],
                            hT[b][128 * et : 128 * (et + 1), :],
                        )

                # ---- attention per batch ----
                for gi in range(G):
                    b = blk * G + gi
                    pr, half = gi // 2, (gi % 2) * L

                    pn = [php.tile([128, E], F32R, tag=f"pn{i}", name=f"pn{i}") for i in range(2)]
                    hn = [php.tile([128, E], F32R, tag=f"hn{i}", name=f"hn{i}") for i in range(2)]
                    for i in range(2):
                        nc.sync.dma_start(pn[i][:], p_nat[b][128 * i : 128 * (i + 1), :])
                        nc.sync.dma_start(hn[i][:], h_nat[b][128 * i : 128 * (i + 1), :])

                    # F_pT/F_hT = tanh(W_F^T @ X^T)  [2 x 128a, 256l]
                    fpt = [attp.tile([128, L], F32R, tag=f"fpt{i}", name=f"fpt{i}") for i in range(2)]
                    fht = [attp.tile([128, L], F32R, tag=f"fht{i}", name=f"fht{i}") for i in range(2)]
                    for at in range(2):
                        accp = q(at, [128, L], "fp_acc")
                        acch = q(2 + at, [128, L], "fh_acc")
                        for et in range(4):
                            lhsT = wf[et][:, 128 * at : 128 * (at + 1)]
                            nc.tensor.matmul(
                                accp[:], lhsT, xt[pr][0][et][:, half : half + L],
                                start=(et == 0), stop=(et == 3),
                            )
                            nc.tensor.matmul(
                                acch[:], lhsT, xt[pr][1][et][:, half : half + L],
                                start=(et == 0), stop=(et == 3),
                            )
                        nc.scalar.activation(fpt[at][:], accp[:], ACT_F.Tanh)
                        nc.scalar.activation(fht[at][:], acch[:], ACT_F.Tanh)

                    # S = F_p @ F_h^T, softmax rows -> attn; attnT via PE transpose
                    attn = [attp.tile([128, L], F32R, tag=f"attn{i}", name=f"attn{i}") for i in range(2)]
                    attnT = [attp.tile([128, L], F32R, tag=f"attnT{i}", name=f"attnT{i}") for i in range(2)]
                    for pt_i in range(2):
                        s_acc = q(pt_i, [128, L], "s_acc")
                        for ak in range(2):
                            nc.tensor.matmul(
                                s_acc[:],
                                fpt[ak][:, 128 * pt_i : 128 * (pt_i + 1)],
                                fht[ak][:],
                                start=(ak == 0), stop=(ak == 1),
                            )
                        negmax = attp.tile([128, 1], F32, tag="negmax", name="negmax")
                        nc.vector.tensor_reduce(
                            negmax[:], s_acc[:], axis=mybir.AxisListType.X,
                            op=mybir.AluOpType.max, negate=True,
                        )
                        exps = attp.tile([128, L], F32, tag="exps", name="exps")
                        denom = attp.tile([128, 1], F32, tag="denom", name="denom")
                        nc.scalar.activation(
                            exps[:], s_acc[:], ACT_F.Exp,
                            bias=negmax[:], accum_out=denom[:],
                        )
                        recip = attp.tile([128, 1], F32, tag="recip", name="recip")
                        nc.vector.reciprocal(recip[:], denom[:])
                        nc.vector.tensor_scalar_mul(attn[pt_i][:], exps[:], recip[:])
                    for i in range(2):
                        for j in range(2):
                            tp = qr(2 + j, [128, 128], "tp")
                            nc.tensor.transpose(
                                tp[:], attn[i][:, 128 * j : 128 * (j + 1)], idt[:]
                            )
                            nc.vector.tensor_copy(
                                attnT[j][:, 128 * i : 128 * (i + 1)], tp[:]
                            )

                    # betasT[e,l] / alphasT[e,h] -> xt k-tiles 4..7
                    for et in range(4):
                        b_acc = q(0, [128, L], "b_acc")
                        a_acc = q(1, [128, L], "a_acc")
                        for k in range(2):
                            nc.tensor.matmul(
                                b_acc[:], hn[k][:, 128 * et : 128 * (et + 1)],
                                attnT[k][:], start=(k == 0), stop=(k == 1),
                            )
                            nc.tensor.matmul(
                                a_acc[:], pn[k][:, 128 * et : 128 * (et + 1)],
                                attn[k][:], start=(k == 0), stop=(k == 1),
                            )
                        nc.vector.tensor_copy(xt[pr][0][4 + et][:, half : half + L], b_acc[:])
                        nc.vector.tensor_copy(xt[pr][1][4 + et][:, half : half + L], a_acc[:])

                # ---- compare GEMM + aggregate ----
                for fm in range(NMT_G):
                    accs = {}
                    for s in range(2):
                        for pr in range(2):
                            accs[(s, pr)] = q(s * 2 + pr, [128, 2 * L], f"g{s}{pr}")
                    for ck in range(NKT_G):
                        lhsT = wg[ck][:, 128 * fm : 128 * (fm + 1)]
                        for s in range(2):
                            for pr in range(2):
                                nc.tensor.matmul(
                                    accs[(s, pr)][:], lhsT, xt[pr][s][ck][:],
                                    start=(ck == 0), stop=(ck == NKT_G - 1),
                                )
                    for s in range(2):
                        for pr in range(2):
                            for half_i in range(2):
                                gi = pr * 2 + half_i
                                b = blk * G + gi
                                nc.scalar.activation(
                                    scrap[:],
                                    accs[(s, pr)][:, half_i * L : (half_i + 1) * L],
                                    ACT_F.Tanh,
                                    accum_out=vT[s * NMT_G + fm][:, b : b + 1],
                                )

            # ================= final MLP =================
            ones_bf = mlpp.tile([1, BL], BF16, tag="ones_bf", name="ones_bf")
            nc.sync.dma_start(ones_bf[:], ones_bf_d[:])
            ones_r = mlpp.tile([1, BL], F32R, tag="ones_r", name="ones_r")
            nc.sync.dma_start(ones_r[:], ones_r_d[:])

            vtb = [mlpp.tile([128, BL], BF16, tag=f"vtb{i}", name=f"vtb{i}") for i in range(32)]
            for i in range(32):
                nc.vector.tensor_copy(vtb[i][:], vT[i][:])

            # a1 = tanh(v @ W1 + b1): psum [BL, 512] x4
            a1 = mlpp.tile([BL, FF], F32R, tag="a1", name="a1")
            a1_accs = [q(n, [BL, 512], f"a1acc{n}") for n in range(4)]
            for kt in range(33):
                band = w1sp.tile([128, FF], BF16, tag="wband", name="w1band")
                rows = min(128, 2 * FF + 1 - 128 * kt)
                nc.sync.dma_start(band[:rows, :], w1_d[128 * kt : 128 * kt + rows, :])
                lhsT = vtb[kt][:] if kt < 32 else ones_bf[:]
                for n in range(4):
                    nc.tensor.matmul(
                        a1_accs[n][:], lhsT, band[:rows, 512 * n : 512 * (n + 1)],
                        start=(kt == 0), stop=(kt == 32),
                    )
            for n in range(4):
                nc.scalar.activation(
                    a1[:, 512 * n : 512 * (n + 1)], a1_accs[n][:], ACT_F.Tanh
                )

            # a1T in bf16 via PE transpose
            a1tb = [mlpp.tile([128, BL], BF16, tag=f"a1tb{i}", name=f"a1tb{i}") for i in range(16)]
            for i in range(16):
                tp = qr(0, [128, BL], "tpa1")
                nc.tensor.transpose(
                    tp[:], a1[:, 128 * i : 128 * (i + 1)], idt[:BL, :BL]
                )
                nc.vector.tensor_copy(a1tb[i][:], tp[:])

            # a2 = tanh(a1 @ W2 + b2)
            a2 = mlpp.tile([BL, FF], F32R, tag="a2", name="a2")
            a2_accs = [q(n, [BL, 512], f"a2acc{n}") for n in range(4)]
            for kt in range(17):
                band = w1sp.tile([128, FF], BF16, tag="wband", name="w2band")
                rows = min(128, FF + 1 - 128 * kt)
                nc.sync.dma_start(band[:rows, :], w2_d[128 * kt : 128 * kt + rows, :])
                lhsT = a1tb[kt][:] if kt < 16 else ones_bf[:]
                for n in range(4):
                    nc.tensor.matmul(
                        a2_accs[n][:], lhsT, band[:rows, 512 * n : 512 * (n + 1)],
                        start=(kt == 0), stop=(kt == 16),
                    )
            for n in range(4):
                nc.scalar.activation(
                    a2[:, 512 * n : 512 * (n + 1)], a2_accs[n][:], ACT_F.Tanh
                )

            # a2T (f32r) + out = a2 @ W3 + b3
            a2t = [mlpp.tile([128, BL], F32R, tag=f"a2t{i}", name=f"a2t{i}") for i in range(16)]
            for i in range(16):
                tp = qr(1, [128, BL], "tpa2")
                nc.tensor.transpose(
                    tp[:], a2[:, 128 * i : 128 * (i + 1)], idt[:BL, :BL]
                )
                nc.vector.tensor_copy(a2t[i][:], tp[:])

            o_acc = q(2, [BL, 4], "o_acc")
            for kt in range(16):
                nc.tensor.matmul(
                    o_acc[:], a2t[kt][:], w3t[:, 4 * kt : 4 * kt + 4],
                    start=(kt == 0), stop=False,
                )
            # bias row: K=1 with ones lhsT against w3 packed cols 64..67 row 0
            nc.tensor.matmul(
                o_acc[:], ones_r[:], w3t[0:1, 64:68], start=False, stop=True
            )
            out_s = mlpp.tile([BL, 3], F32, tag="out_s", name="out_s")
            nc.vector.tensor_copy(out_s[:], o_acc[:, 0:3])
            nc.sync.dma_start(out_d[:], out_s[:])

    _split_multiwait(nc)
    return nc


_NC_CACHE = None


def _get_nc():
    global _NC_CACHE
    if _NC_CACHE is None:
        _NC_CACHE = build_nc()
    return _NC_CACHE


def _prep_in_maps(premises, hypotheses, W_F, W_G, W1, b1, W2, b2, W3, b3):
    premises = np.asarray(premises, dtype=np.float32)
    hypotheses = np.asarray(hypotheses, dtype=np.float32)
    pT = np.ascontiguousarray(premises.transpose(0, 2, 1))
    hT = np.ascontiguousarray(hypotheses.transpose(0, 2, 1))

    wf = np.asarray(W_F, dtype=np.float32)
    wg = np.asarray(W_G, dtype=np.float32)
    w1 = np.concatenate(
        [np.asarray(W1, np.float32), np.asarray(b1, np.float32)[None, :]], axis=0
    ).astype(ml_dtypes.bfloat16)
    w2 = np.concatenate(
        [np.asarray(W2, np.float32), np.asarray(b2, np.float32)[None, :]], axis=0
    ).astype(ml_dtypes.bfloat16)
    # w3 packed: [128, 51]; col block kt (3 cols) = rows kt*128..kt*128+127 of W3
    w3a = np.asarray(W3, np.float32)
    b3a = np.asarray(b3, np.float32)
    w3p = np.zeros((128, 68), dtype=np.float32)
    for kt in range(16):
        w3p[:, 4 * kt : 4 * kt + 3] = w3a[128 * kt : 128 * (kt + 1), :]
    w3p[0, 64:67] = b3a
    ident = np.eye(128, dtype=np.float32)

    in_maps = []
    for c in range(N_CORES):
        sl = slice(c * BL, (c + 1) * BL)
        in_maps.append(
            {
                "p_nat": np.ascontiguousarray(premises[sl]),
                "h_nat": np.ascontiguousarray(hypotheses[sl]),
                "pT": np.ascontiguousarray(pT[sl]),
                "hT": np.ascontiguousarray(hT[sl]),
                "wf": wf,
                "wg": wg,
                "w1": w1,
                "w2": w2,
                "w3": w3p,
                "ident": ident,
                "ones_bf": np.ones((1, BL), dtype=ml_dtypes.bfloat16),
                "ones_r": np.ones((1, BL), dtype=np.float32),
            }
        )
    return in_maps


def _run(inputs, trace=False):
    nc = _get_nc()
    in_maps = _prep_in_maps(**inputs)
    res = run_bass_kernel_spmd(
        nc, in_maps, core_ids=list(range(N_CORES)), trace=trace
    )
    out = np.concatenate(
        [np.asarray(res.results[c]["out"]) for c in range(N_CORES)], axis=0
    )
    return out.astype(np.float32), res


def kernel(**inputs):
    out, _ = _run(inputs, trace=False)
    return out


# revision 6
# speedup vs baseline: 1.1045x; 1.1045x over previous
"""Trainium2 Bass kernel for the decomposable-attention alignment model.

Data-parallel over batch across 8 NeuronCores (16 batch elements each).
All matmuls run as float32r (TF32-like: 1 cycle/row at free-dim>=256,
~1e-4 relative error). The compare GEMM packs two batch elements per
moving operand (N=512). Aggregation (sum over length after tanh) is
fused into the PSUM-drain activation via accum_out. The final MLP
streams W1/W2 in bf16 with biases folded in as extra contraction rows.
"""

import sys

sys.path.insert(0, "/opt/trn_rl_repo")

import numpy as np
import ml_dtypes

from concourse import bass, tile, mybir
from concourse.bass_utils import run_bass_kernel_spmd

F32 = mybir.dt.float32
F32R = mybir.dt.float32r
BF16 = mybir.dt.bfloat16

N_CORES = 8
B, L, E, A, FF = 128, 256, 512, 256, 2048
BL = B // N_CORES  # 16 batch elements per core
G = 4  # batch block size (pairs of 2)
ACT_F = mybir.ActivationFunctionType


def _split_multiwait(nc, max_waits=1):
    """walrus in this image rejects >1 sync wait per instruction; hoist
    extras onto InstNoOp placed just before the offender."""
    for f in nc.m.functions:
        for bb in f.blocks:
            out = []
            changed = False
            for inst in bb.instructions:
                si = inst.sync_info
                waits = list(si.on_wait) if (si is not None and si.on_wait) else []
                if len(waits) > max_waits:
                    extra, keep = waits[:-max_waits], waits[-max_waits:]
                    for j, w in enumerate(extra):
                        nop = mybir.InstNoOp(name=f"{inst.name}-ws{j}")
                        nop.engine = inst.engine
                        nop.sync_info = mybir.SyncInfo(on_wait=[w], on_update=[])
                        out.append(nop)
                    inst.sync_info = mybir.SyncInfo(
                        on_wait=keep, on_update=list(si.on_update or [])
                    )
                    changed = True
                out.append(inst)
            if changed:
                bb.instructions = out


def build_nc():
    nc = bass.Bass()

    p_nat = nc.declare_dram_parameter("p_nat", [BL, L, E], F32R, isOutput=False)
    h_nat = nc.declare_dram_parameter("h_nat", [BL, L, E], F32R, isOutput=False)
    pT = nc.declare_dram_parameter("pT", [BL, E, L], F32R, isOutput=False)
    hT = nc.declare_dram_parameter("hT", [BL, E, L], F32R, isOutput=False)
    pTb = nc.declare_dram_parameter("pTb", [BL, E, L], BF16, isOutput=False)
    hTb = nc.declare_dram_parameter("hTb", [BL, E, L], BF16, isOutput=False)
    wf_d = nc.declare_dram_parameter("wf", [E, A], F32R, isOutput=False)
    wg_d = nc.declare_dram_parameter("wg", [2 * E, FF], BF16, isOutput=False)
    w1_d = nc.declare_dram_parameter("w1", [2 * FF + 1, FF], BF16, isOutput=False)
    w2_d = nc.declare_dram_parameter("w2", [FF + 1, FF], BF16, isOutput=False)
    w3_d = nc.declare_dram_parameter("w3", [128, 68], F32R, isOutput=False)
    ident_d = nc.declare_dram_parameter("ident", [128, 128], F32R, isOutput=False)
    ones_bf_d = nc.declare_dram_parameter("ones_bf", [1, BL], BF16, isOutput=False)
    ones_r_d = nc.declare_dram_parameter("ones_r", [1, BL], F32R, isOutput=False)
    out_d = nc.declare_dram_parameter("out", [BL, 3], F32, isOutput=True)

    NKT_G = 2 * E // 128  # 8 contraction tiles for the compare GEMM
    NMT_G = FF // 128  # 16 output tiles

    with tile.TileContext(nc) as tc:
        with (
            tc.tile_pool(name="wpool", bufs=1) as wpool,
            tc.tile_pool(name="xt", bufs=1) as xtp,
            tc.tile_pool(name="ph", bufs=1) as php,
            tc.tile_pool(name="att", bufs=2) as attp,
            tc.tile_pool(name="vt", bufs=1) as vtp,
            tc.tile_pool(name="mlp", bufs=1) as mlpp,
            tc.tile_pool(name="w1s", bufs=8) as w1sp,
            tc.tile_pool(name="scrap", bufs=3) as scrapp,
            tc.tile_pool(name="psum", bufs=2, space="PSUM") as psp,
        )        :
            # ---- resident weights ----
            wg = [wpool.tile([128, FF], BF16, tag=f"wg{k}", name=f"wg{k}") for k in range(NKT_G)]
            for k in range(NKT_G):
                nc.sync.dma_start(wg[k][:], wg_d[128 * k : 128 * (k + 1), :])
            wf = [wpool.tile([128, A], F32R, tag=f"wf{k}", name=f"wf{k}") for k in range(4)]
            for k in range(4):
                nc.sync.dma_start(wf[k][:], wf_d[128 * k : 128 * (k + 1), :])
            idt = wpool.tile([128, 128], F32R, tag="idt", name="idt")
            nc.sync.dma_start(idt[:], ident_d[:])
            w3t = wpool.tile([128, 68], F32R, tag="w3t", name="w3t")
            nc.sync.dma_start(w3t[:], w3_d[:])

            # vT accumulator: 32 tiles [128, BL] fp32, c4 index = side*16 + fm
            vT = [vtp.tile([128, BL], F32, tag=f"vt{i}", name=f"vt{i}") for i in range(32)]



            def q(i, shape, name):
                return psp.tile(shape, F32, tag=f"q{i}", name=name)

            def qr(i, shape, name):
                return psp.tile(shape, F32R, tag=f"q{i}", name=name)

            # ================= batch blocks =================
            for blk in range(BL // G):
                # xt[pair][side][ck]: [128, 512]; cols 0:256 = batch 2p,
                # cols 256: = batch 2p+1. ck 0..3 from pT/hT, 4..7 = betasT/alphasT
                xt = [
                    [
                        [
                            xtp.tile(
                                [128, 2 * L], BF16,
                                tag=f"xt{p}_{s}_{ck}", name=f"xt{p}_{s}_{ck}",
                            )
                            for ck in range(NKT_G)
                        ]
                        for s in range(2)
                    ]
                    for p in range(2)
                ]
                for gi in range(G):
                    b = blk * G + gi
                    pr, half = gi // 2, (gi % 2) * L
                    for et in range(4):
                        nc.sync.dma_start(
                            xt[pr][0][et][:, half : half + L],
                            pTb[b][128 * et : 128 * (et + 1), :],
                        )
                        nc.sync.dma_start(
                            xt[pr][1][et][:, half : half + L],
                            hTb[b][128 * et : 128 * (et + 1), :],
                        )

                # ---- attention, per pair of batches ----
                for pr in range(2):
                    # pair-packed transposed loads (f32r) for the F matmuls
                    ptf = [php.tile([128, 2 * L], F32R, tag=f"ptf{i}", name=f"ptf{i}") for i in range(4)]
                    htf = [php.tile([128, 2 * L], F32R, tag=f"htf{i}", name=f"htf{i}") for i in range(4)]
                    for gi2 in range(2):
                        b = blk * G + pr * 2 + gi2
                        for et in range(4):
                            nc.sync.dma_start(
                                ptf[et][:, gi2 * L : (gi2 + 1) * L],
                                pT[b][128 * et : 128 * (et + 1), :],
                            )
                            nc.sync.dma_start(
                                htf[et][:, gi2 * L : (gi2 + 1) * L],
                                hT[b][128 * et : 128 * (et + 1), :],
                            )

                    # F_pT/F_hT for both batches of the pair: [2 x 128a, 512]
                    fpt = [attp.tile([128, 2 * L], F32R, tag=f"fpt{i}", name=f"fpt{i}") for i in range(2)]
                    fht = [attp.tile([128, 2 * L], F32R, tag=f"fht{i}", name=f"fht{i}") for i in range(2)]
                    for at in range(2):
                        accp = q(at, [128, 2 * L], "fp_acc")
                        acch = q(2 + at, [128, 2 * L], "fh_acc")
                        for et in range(4):
                            lhsT = wf[et][:, 128 * at : 128 * (at + 1)]
                            nc.tensor.matmul(
                                accp[:], lhsT, ptf[et][:],
                                start=(et == 0), stop=(et == 3),
                            )
                            nc.tensor.matmul(
                                acch[:], lhsT, htf[et][:],
                                start=(et == 0), stop=(et == 3),
                            )
                        nc.scalar.activation(fpt[at][:], accp[:], ACT_F.Tanh)
                        nc.scalar.activation(fht[at][:], acch[:], ACT_F.Tanh)

                    for gi2 in range(2):
                        gi = pr * 2 + gi2
                        b = blk * G + gi
                        half = gi2 * L

                        pn = [php.tile([128, E], F32R, tag=f"pn{i}", name=f"pn{i}") for i in range(2)]
                        hn = [php.tile([128, E], F32R, tag=f"hn{i}", name=f"hn{i}") for i in range(2)]
                        for i in range(2):
                            nc.sync.dma_start(pn[i][:], p_nat[b][128 * i : 128 * (i + 1), :])
                            nc.sync.dma_start(hn[i][:], h_nat[b][128 * i : 128 * (i + 1), :])

                        # S = F_p @ F_h^T, softmax rows -> attn; attnT via PE transpose
                        attn = [attp.tile([128, L], F32R, tag=f"attn{i}", name=f"attn{i}") for i in range(2)]
                        attnT = [attp.tile([128, L], F32R, tag=f"attnT{i}", name=f"attnT{i}") for i in range(2)]
                        for pt_i in range(2):
                            s_acc = q(2 + pt_i, [128, L], "s_acc")
                            for ak in range(2):
                                nc.tensor.matmul(
                                    s_acc[:],
                                    fpt[ak][:, half + 128 * pt_i : half + 128 * (pt_i + 1)],
                                    fht[ak][:, half : half + L],
                                    start=(ak == 0), stop=(ak == 1),
                                )
                            negmax = attp.tile([128, 1], F32, tag="negmax", name="negmax")
                            nc.vector.tensor_reduce(
                                negmax[:], s_acc[:], axis=mybir.AxisListType.X,
                                op=mybir.AluOpType.max, negate=True,
                            )
                            exps = attp.tile([128, L], F32, tag="exps", name="exps")
                            denom = attp.tile([128, 1], F32, tag="denom", name="denom")
                            nc.scalar.activation(
                                exps[:], s_acc[:], ACT_F.Exp,
                                bias=negmax[:], accum_out=denom[:],
                            )
                            recip = attp.tile([128, 1], F32, tag="recip", name="recip")
                            nc.vector.reciprocal(recip[:], denom[:])
                            nc.vector.tensor_scalar_mul(attn[pt_i][:], exps[:], recip[:])
                        for i in range(2):
                            for j in range(2):
                                tp = qr(2 + j, [128, 128], "tp")
                                nc.tensor.transpose(
                                    tp[:], attn[i][:, 128 * j : 128 * (j + 1)], idt[:]
                                )
                                nc.vector.tensor_copy(
                                    attnT[j][:, 128 * i : 128 * (i + 1)], tp[:]
                                )

                        # betasT[e,l] / alphasT[e,h] -> xt k-tiles 4..7 (bf16 cast)
                        for et in range(4):
                            b_acc = q(0, [128, L], "b_acc")
                            a_acc = q(1, [128, L], "a_acc")
                            for k in range(2):
                                nc.tensor.matmul(
                                    b_acc[:], hn[k][:, 128 * et : 128 * (et + 1)],
                                    attnT[k][:], start=(k == 0), stop=(k == 1),
                                )
                                nc.tensor.matmul(
                                    a_acc[:], pn[k][:, 128 * et : 128 * (et + 1)],
                                    attn[k][:], start=(k == 0), stop=(k == 1),
                                )
                            nc.vector.tensor_copy(xt[pr][0][4 + et][:, half : half + L], b_acc[:])
                            nc.vector.tensor_copy(xt[pr][1][4 + et][:, half : half + L], a_acc[:])

                # ---- compare GEMM + aggregate ----
                for fm in range(NMT_G):
                    accs = {}
                    for s in range(2):
                        for pr in range(2):
                            accs[(s, pr)] = q(s * 2 + pr, [128, 2 * L], f"g{s}{pr}")
                    for ck in range(NKT_G):
                        lhsT = wg[ck][:, 128 * fm : 128 * (fm + 1)]
                        for s in range(2):
                            for pr in range(2):
                                nc.tensor.matmul(
                                    accs[(s, pr)][:], lhsT, xt[pr][s][ck][:],
                                    start=(ck == 0), stop=(ck == NKT_G - 1),
                                )
                    for s in range(2):
                        for pr in range(2):
                            scrap = scrapp.tile([128, 2 * L], BF16, tag="scrap", name="scrap")
                            nc.scalar.activation(
                                scrap[:], accs[(s, pr)][:], ACT_F.Tanh
                            )
                            for half_i in range(2):
                                gi = pr * 2 + half_i
                                b = blk * G + gi
                                nc.vector.tensor_reduce(
                                    vT[s * NMT_G + fm][:, b : b + 1],
                                    scrap[:, half_i * L : (half_i + 1) * L],
                                    axis=mybir.AxisListType.X,
                                    op=mybir.AluOpType.add,
                                )

            # ================= final MLP =================
            ones_bf = mlpp.tile([1, BL], BF16, tag="ones_bf", name="ones_bf")
            nc.sync.dma_start(ones_bf[:], ones_bf_d[:])
            ones_r = mlpp.tile([1, BL], F32R, tag="ones_r", name="ones_r")
            nc.sync.dma_start(ones_r[:], ones_r_d[:])

            vtb = [mlpp.tile([128, BL], BF16, tag=f"vtb{i}", name=f"vtb{i}") for i in range(32)]
            for i in range(32):
                nc.vector.tensor_copy(vtb[i][:], vT[i][:])

            # a1 = tanh(v @ W1 + b1): psum [BL, 512] x4
            a1 = mlpp.tile([BL, FF], F32R, tag="a1", name="a1")
            a1_accs = [q(n, [BL, 512], f"a1acc{n}") for n in range(4)]
            for kt in range(33):
                band = w1sp.tile([128, FF], BF16, tag="wband", name="w1band")
                rows = min(128, 2 * FF + 1 - 128 * kt)
                nc.sync.dma_start(band[:rows, :], w1_d[128 * kt : 128 * kt + rows, :])
                lhsT = vtb[kt][:] if kt < 32 else ones_bf[:]
                for n in range(4):
                    nc.tensor.matmul(
                        a1_accs[n][:], lhsT, band[:rows, 512 * n : 512 * (n + 1)],
                        start=(kt == 0), stop=(kt == 32),
                    )
            for n in range(4):
                nc.scalar.activation(
                    a1[:, 512 * n : 512 * (n + 1)], a1_accs[n][:], ACT_F.Tanh
                )

            # a1T in bf16 via PE transpose
            a1tb = [mlpp.tile([128, BL], BF16, tag=f"a1tb{i}", name=f"a1tb{i}") for i in range(16)]
            for i in range(16):
                tp = qr(0, [128, BL], "tpa1")
                nc.tensor.transpose(
                    tp[:], a1[:, 128 * i : 128 * (i + 1)], idt[:BL, :BL]
                )
                nc.vector.tensor_copy(a1tb[i][:], tp[:])

            # a2 = tanh(a1 @ W2 + b2)
            a2 = mlpp.tile([BL, FF], F32R, tag="a2", name="a2")
            a2_accs = [q(n, [BL, 512], f"a2acc{n}") for n in range(4)]
            for kt in range(17):
                band = w1sp.tile([128, FF], BF16, tag="wband", name="w2band")
                rows = min(128, FF + 1 - 128 * kt)
                nc.sync.dma_start(band[:rows, :], w2_d[128 * kt : 128 * kt + rows, :])
                lhsT = a1tb[kt][:] if kt < 16 else ones_bf[:]
                for n in range(4):
                    nc.tensor.matmul(
                        a2_accs[n][:], lhsT, band[:rows, 512 * n : 512 * (n + 1)],
                        start=(kt == 0), stop=(kt == 16),
                    )
            for n in range(4):
                nc.scalar.activation(
                    a2[:, 512 * n : 512 * (n + 1)], a2_accs[n][:], ACT_F.Tanh
                )

            # a2T (f32r) + out = a2 @ W3 + b3
            a2t = [mlpp.tile([128, BL], F32R, tag=f"a2t{i}", name=f"a2t{i}") for i in range(16)]
            for i in range(16):
                tp = qr(1, [128, BL], "tpa2")
                nc.tensor.transpose(
                    tp[:], a2[:, 128 * i : 128 * (i + 1)], idt[:BL, :BL]
                )
                nc.vector.tensor_copy(a2t[i][:], tp[:])

            o_acc = q(2, [BL, 4], "o_acc")
            for kt in range(16):
                nc.tensor.matmul(
                    o_acc[:], a2t[kt][:], w3t[:, 4 * kt : 4 * kt + 4],
                    start=(kt == 0), stop=False,
                )
            # bias row: K=1 with ones lhsT against w3 packed cols 64..67 row 0
            nc.tensor.matmul(
                o_acc[:], ones_r[:], w3t[0:1, 64:68], start=False, stop=True
            )
            out_s = mlpp.tile([BL, 3], F32, tag="out_s", name="out_s")
            nc.vector.tensor_copy(out_s[:], o_acc[:, 0:3])
            nc.sync.dma_start(out_d[:], out_s[:])

    _split_multiwait(nc)
    return nc


_NC_CACHE = None


def _get_nc():
    global _NC_CACHE
    if _NC_CACHE is None:
        _NC_CACHE = build_nc()
    return _NC_CACHE


def _prep_in_maps(premises, hypotheses, W_F, W_G, W1, b1, W2, b2, W3, b3):
    premises = np.asarray(premises, dtype=np.float32)
    hypotheses = np.asarray(hypotheses, dtype=np.float32)
    pT = np.ascontiguousarray(premises.transpose(0, 2, 1))
    hT = np.ascontiguousarray(hypotheses.transpose(0, 2, 1))

    wf = np.asarray(W_F, dtype=np.float32)
    wg = np.asarray(W_G, dtype=np.float32).astype(ml_dtypes.bfloat16)
    w1 = np.concatenate(
        [np.asarray(W1, np.float32), np.asarray(b1, np.float32)[None, :]], axis=0
    ).astype(ml_dtypes.bfloat16)
    w2 = np.concatenate(
        [np.asarray(W2, np.float32), np.asarray(b2, np.float32)[None, :]], axis=0
    ).astype(ml_dtypes.bfloat16)
    # w3 packed: [128, 51]; col block kt (3 cols) = rows kt*128..kt*128+127 of W3
    w3a = np.asarray(W3, np.float32)
    b3a = np.asarray(b3, np.float32)
    w3p = np.zeros((128, 68), dtype=np.float32)
    for kt in range(16):
        w3p[:, 4 * kt : 4 * kt + 3] = w3a[128 * kt : 128 * (kt + 1), :]
    w3p[0, 64:67] = b3a
    ident = np.eye(128, dtype=np.float32)

    in_maps = []
    for c in range(N_CORES):
        sl = slice(c * BL, (c + 1) * BL)
        in_maps.append(
            {
                "p_nat": np.ascontiguousarray(premises[sl]),
                "h_nat": np.ascontiguousarray(hypotheses[sl]),
                "pT": np.ascontiguousarray(pT[sl]),
                "hT": np.ascontiguousarray(hT[sl]),
                "pTb": np.ascontiguousarray(pT[sl]).astype(ml_dtypes.bfloat16),
                "hTb": np.ascontiguousarray(hT[sl]).astype(ml_dtypes.bfloat16),
                "wf": wf,
                "wg": wg,
                "w1": w1,
                "w2": w2,
                "w3": w3p,
                "ident": ident,
                "ones_bf": np.ones((1, BL), dtype=ml_dtypes.bfloat16),
                "ones_r": np.ones((1, BL), dtype=np.float32),
            }
        )
    return in_maps


def _run(inputs, trace=False):
    nc = _get_nc()
    in_maps = _prep_in_maps(**inputs)
    res = run_bass_kernel_spmd(
        nc, in_maps, core_ids=list(range(N_CORES)), trace=trace
    )
    out = np.concatenate(
        [np.asarray(res.results[c]["out"]) for c in range(N_CORES)], axis=0
    )
    return out.astype(np.float32), res


def kernel(**inputs):
    out, _ = _run(inputs, trace=False)
    return out
